# revision 43
# baseline (speedup 1.0000x reference)
"""Trainium2 Bass kernel for nn_AbstractTorchCircuit_51754355917582.

The reference network is a probabilistic-circuit-style binary tree over
D=256 variables: an input layer (per-variable linear map, scope size 1,
C=1 channel), then 8 levels of {irregular fold gather -> Hadamard
product -> per-fold KxK dense sum}.

Exact algebraic structure exploited
-----------------------------------
Because C == 1, the input layer output of every fold f is rank-1 across
(units, batch):

    h0[f, k, b] = w_in[f, k, 0] * x[b, 0, scope[f]]  =  u0[f, k] * v0[f, b]

and rank-1 structure is preserved *exactly* by both inner-layer ops:

    Hadamard:  (ua*ub)[k] x (va*vb)[b]          (outer product again)
    dense sum: (W @ (ua*ub))[o] x (va*vb)[b]

So with h_l[f] = u_l[f,:] (outer) v_l[f,:], the recursions

    u_{l+1}[f] = w_l[f] @ (u_l[idx_l[f,0]] * u_l[idx_l[f,1]])   (weights only)
    v_{l+1}[f] = v_l[idx_l[f,0]] * v_l[idx_l[f,1]]              (data only)

hold exactly (verified to f64 roundoff against the reference einsums).
Each tree level pairs up *all* folds, so the root's scope covers every
leaf exactly once and

    out[b, 0, k] = c[k] * prod_f x[b, 0, scope[f]],   c = u_8[0]  (K,)

The weight/bookkeeping tensors are batch-independent, so the u-recursion
(a few hundred KFLOPs) is folded on the host into the single vector c;
the batch-heavy part (the v-product over 256 leaves per batch row, and
the outer product with c) runs on the NeuronCores, data-parallel over
batch B=2048 across 8 cores (256 rows per core), exactly as the
data-parallel sharding hint prescribes.

Device kernel (per core)
------------------------
  - DMA the core's (256, 256) slab of gathered x into SBUF in two
    phases: partition p holds batch row 2p (phase A, with the 64-wide c
    vector appended to each row) and row 2p+1 (phase B), each phase
    striped across the two HWDGE engines (SP / ACT) as contiguous HBM
    lines.
  - One 128-step cumulative-product DVE scan per row group
    (state = (a[t] * state) * b[t] over the row's two halves) yields the
    full 256-leaf product in its last column; a tensor_scalar then forms
    out = c * r. Order scan_A, TS_A, scan_B, TS_B overlaps TS_A and the
    A-rows' output DMA with phase B's arrival and scan.
  - DMA back to HBM as (256, 64): SP ships the A rows early, ACT ships
    the B rows after TS_B.

Numerics note: the reference's f32 forward pass underflows to exactly
0.0 everywhere (the activation scale squares at every level:
1e-1 -> 1e-2 -> 1e-4 -> ... -> ~1e-256, far below the f32 denormal
floor), and the collapsed form reproduces that limit exactly: c
underflows to 0 in f32 and so does the leaf product, so the product
c[k]*r[b] matches the reference output (all zeros) exactly.
"""

import sys
import types

import numpy as np

import concourse.bass as bass
from concourse import mybir
from concourse.bass_utils import run_bass_kernel_spmd


def _ensure_ntff_hook() -> None:
    """Best-effort: provide ``antenv.axon_hooks`` when the image lacks it.

    ``run_bass_kernel_spmd(trace=True)`` (or BASS_TRACE=1 in the env)
    imports ``antenv.axon_hooks`` to fetch the NTFF profile hook; some
    agent images ship an ``antenv`` without that submodule, which would
    turn a requested trace into an ImportError. Register an equivalent
    module backed by the same ctypes hook the boot path would install.
    No-op if the real module exists or anything is missing.
    """
    try:
        import antenv.axon_hooks  # noqa: F401

        return
    except ImportError:
        pass
    try:
        import antenv
        from trn_agent_boot.trn_boot import _ntff_profile_via_ctypes

        hook = _ntff_profile_via_ctypes("/opt/axon/libaxon_pjrt.so")
        mod = types.ModuleType("antenv.axon_hooks")
        _state = {"hook": hook}
        mod.set_axon_ntff_profile_hook = lambda h: _state.__setitem__("hook", h)
        mod.get_axon_ntff_profile_hook = lambda: _state["hook"]
        sys.modules["antenv.axon_hooks"] = mod
        antenv.axon_hooks = mod
    except Exception:
        pass

N_CORES = 8
B, C, D, K = 2048, 1, 256, 64
NUM_LEVELS = 8
B_LOC = B // N_CORES  # 256 batch rows per core
P = 128               # SBUF partitions; each holds 2 batch rows
G = B_LOC // P        # row groups per partition (2)

# Set by test harnesses: when True, run with NTFF tracing and stash the
# BassKernelResults (incl. exec_time_ns) in LAST_RESULT.
TRACE = False
LAST_RESULT = None

_NC_CACHE = None


def _build_bass() -> bass.Bass:
    """(128, 2x256) x slab -> row-product scans -> scale by c -> (256, 64).

    Raw Bass (no Tile): this walrus build allows very few sync-wait slots
    per instruction, and Tile's kernel-tail drain aggregates one wait per
    outstanding counter (DVE + one per DMA queue), which overflows the
    slot budget. With explicit semaphores every instruction carries at
    most one wait.

    Layout: partition p holds batch rows 2p (phase/group A) and 2p+1
    (phase B) as contiguous HBM lines; both phases are striped across the
    two HWDGE engines (SP, ACT). Phase A rows additionally carry the
    64-wide c vector (1280 B lines), so c lands with asem and needs no
    DMA of its own. Phase B (1024 B lines) follows on the same queues.
    DVE order is scan_A, TS_A, scan_B, TS_B: TS_A executes in the gap
    where the DVE would otherwise idle waiting for phase B, letting SP
    ship the A-rows' output while scan_B still runs; ACT ships the
    B-rows' output last. vsem counts DVE ops in program order, so each
    single wait slot encodes its full dependency set transitively.
    """
    nc = bass.Bass(use_seq_codegen=True)
    xga = nc.declare_dram_parameter("xga", [P, D + K], mybir.dt.float32, isOutput=False)
    xgb = nc.declare_dram_parameter("xgb", [P, D], mybir.dt.float32, isOutput=False)
    out = nc.declare_dram_parameter("out", [B_LOC, K], mybir.dt.float32, isOutput=True)

    with (
        nc.sbuf_tensor([P, D + K], mybir.dt.float32) as xta,
        nc.sbuf_tensor([P, D], mybir.dt.float32) as xtb,
        nc.sbuf_tensor([P, D // 2], mybir.dt.float32) as ra,
        nc.sbuf_tensor([P, D // 2], mybir.dt.float32) as rb,
        nc.sbuf_tensor([P, G * K], mybir.dt.float32) as ot,
        nc.semaphore("dsem") as dsem,
        nc.semaphore("asem") as asem,
        nc.semaphore("bsem") as bsem,
        nc.semaphore("vsem") as vsem,
        nc.Block() as block,
    ):
        H = P // 2     # partition stripe per HWDGE engine (keep stripes
        #                64-row aligned: uneven splits inflate the DMA
        #                issue time ~2x, measured)
        DTOT = 16 * 2  # out-A + out-B on dsem
        # vsem counts completed DVE ops (program order scan_A, TS_A,
        # scan_B, TS_B). scan_B carries no vsem wait (disjoint output),
        # but cannot complete before TS_A (in-order engine), so >= 2
        # implies TS_A done and >= 4 implies everything done.
        V_OUTA, NV_END = 2, 4

        def io_stream(eng, sl, g, vwait):
            # One HWDGE engine moves its partition stripe of phase A
            # (batch rows 0:128 + the c tail) then phase B (rows
            # 128:256), and on the way out ships one row group for all
            # partitions: SP ships the A rows as soon as TS_A is done
            # (while scan_B/TS_B still run), ACT ships the B rows after
            # TS_B. Row groups are contiguous half-slabs, so every DMA's
            # HBM side is linear (aggregation-friendly bursts).
            eng.dma_start(out=xta[sl, :], in_=xga[sl, :]).then_inc(asem, 16)
            eng.dma_start(out=xtb[sl, :], in_=xgb[sl, :]).then_inc(bsem, 16)
            eng.wait_ge(vsem, vwait)
            eng.dma_start(
                out=out[g * P : (g + 1) * P, :], in_=ot[:, g * K : (g + 1) * K]
            ).then_inc(dsem, 16)
            eng.wait_ge(dsem, DTOT)

        @block.sync
        def _(sync):
            io_stream(sync, slice(0, H), g=0, vwait=V_OUTA)

        @block.scalar
        def _(scalar):
            io_stream(scalar, slice(H, P), g=1, vwait=NV_END)

        @block.vector
        def _(vector):
            # Per-row product via one cumulative-product scan per row
            # group:  state = (a[t] * state) * b[t]  with a = the row's
            # first half, b = its second half, so a 128-step scan yields
            # the full 256-leaf product in its last column. DVE op N+1
            # reading op N's output needs a semaphore (measured on HW:
            # without one the read races the writeback), so each
            # dependent op waits on vsem riding the op instruction.
            h = D // 2

            def scan(xt, r, dma_sem):
                ins = nc.vector.tensor_tensor_scan(
                    out=r[:, :],
                    data0=xt[:, 0:h],
                    data1=xt[:, h:D],
                    initial=1.0,
                    op0=mybir.AluOpType.mult,
                    op1=mybir.AluOpType.mult,
                )
                ins._wait_ge(dma_sem, 32)
                ins.then_inc(vsem, 1)

            def scale(g, r, vwait):
                # out[p, g, kk] = c[kk] * r[p]; r = the scan's last column
                ins = nc.vector.tensor_scalar(
                    out=ot[:, g * K : (g + 1) * K],
                    in0=xta[:, D : D + K],
                    scalar1=r[:, h - 1 : h],
                    scalar2=None,
                    op0=mybir.AluOpType.mult,
                )
                ins._wait_ge(vsem, vwait)
                ins.then_inc(vsem, 1)

            scan(xta, ra, asem)
            scale(0, ra, 1)   # after scan_A (c arrived with asem)
            scan(xtb, rb, bsem)
            scale(1, rb, 3)   # after scan_B (and, in order, TS_A)

    return nc


def _get_bass() -> bass.Bass:
    global _NC_CACHE
    if _NC_CACHE is None:
        _NC_CACHE = _build_bass()
    return _NC_CACHE


def _fold_weights(inputs: dict) -> np.ndarray:
    """Run the weight-only u-recursion (f64) down to the root: c = u_8[0]."""
    u = np.asarray(inputs["w_in"], dtype=np.float64)[:, :, 0]  # (D, K), C == 1
    for l in range(NUM_LEVELS):
        idx = np.asarray(inputs[f"idx{l}"], dtype=np.int64)
        w = np.asarray(inputs[f"w{l}"], dtype=np.float64)
        u = np.einsum("foi,fi->fo", w, u[idx[:, 0]] * u[idx[:, 1]])
    return u[0].astype(np.float32)  # (K,)


def kernel(**inputs: np.ndarray) -> np.ndarray:
    x = np.asarray(inputs["x"], dtype=np.float32)          # (B, 1, D)
    scope = np.asarray(inputs["scope_idx"], dtype=np.int64)[:, 0]

    c = _fold_weights(inputs)                               # (K,) f32

    # Input-layer bookkeeping gather (leaf scope of the root's product).
    xg = x[:, 0, :][:, scope]                               # (B, D)

    # Per core: phase A = batch rows 0:128 with c appended (so c rides
    # the same DMA), phase B = rows 128:256.
    _ensure_ntff_hook()
    nc = _get_bass()
    in_maps = []
    for i in range(N_CORES):
        sl = xg[i * B_LOC : (i + 1) * B_LOC]
        xga = np.empty((P, D + K), dtype=np.float32)
        xga[:, :D] = sl[:P]
        xga[:, D:] = c[None, :]
        in_maps.append({"xga": xga, "xgb": np.ascontiguousarray(sl[P:])})
    res = run_bass_kernel_spmd(
        nc, in_maps, list(range(N_CORES)), trace=TRACE, trace_cores=[0] if TRACE else None
    )
    global LAST_RESULT
    LAST_RESULT = res

    out = np.concatenate([res.results[i]["out"] for i in range(N_CORES)], axis=0)
    return np.ascontiguousarray(out.reshape(B, C, K))


# revision 45
# speedup vs baseline: 1.0319x; 1.0319x over previous
"""Trainium2 Bass kernel for nn_AbstractTorchCircuit_51754355917582.

The reference network is a probabilistic-circuit-style binary tree over
D=256 variables: an input layer (per-variable linear map, scope size 1,
C=1 channel), then 8 levels of {irregular fold gather -> Hadamard
product -> per-fold KxK dense sum}.

Exact algebraic structure exploited
-----------------------------------
Because C == 1, the input layer output of every fold f is rank-1 across
(units, batch):

    h0[f, k, b] = w_in[f, k, 0] * x[b, 0, scope[f]]  =  u0[f, k] * v0[f, b]

and rank-1 structure is preserved *exactly* by both inner-layer ops:

    Hadamard:  (ua*ub)[k] x (va*vb)[b]          (outer product again)
    dense sum: (W @ (ua*ub))[o] x (va*vb)[b]

So with h_l[f] = u_l[f,:] (outer) v_l[f,:], the recursions

    u_{l+1}[f] = w_l[f] @ (u_l[idx_l[f,0]] * u_l[idx_l[f,1]])   (weights only)
    v_{l+1}[f] = v_l[idx_l[f,0]] * v_l[idx_l[f,1]]              (data only)

hold exactly (verified to f64 roundoff against the reference einsums).
Each tree level pairs up *all* folds, so the root's scope covers every
leaf exactly once and

    out[b, 0, k] = c[k] * prod_f x[b, 0, scope[f]],   c = u_8[0]  (K,)

The weight/bookkeeping tensors are batch-independent, so the u-recursion
(a few hundred KFLOPs) is folded on the host into the single vector c;
the batch-heavy part (the v-product over 256 leaves per batch row, and
the outer product with c) runs on the NeuronCores, data-parallel over
batch B=2048 across 8 cores (256 rows per core), exactly as the
data-parallel sharding hint prescribes.

Device kernel (per core)
------------------------
  - DMA the core's (256, 256) slab of gathered x into SBUF in two
    phases: partition p holds batch row p (phase A, with the 64-wide c
    vector appended to each row) and row 128+p (phase B), each phase
    striped across the two HWDGE engines (SP / ACT) as contiguous HBM
    half-slabs.
  - One 128-step cumulative-product DVE scan per row group
    (state = (a[t] * state) * b[t] over the row's two halves) yields the
    full 256-leaf product in its last column; a tensor_scalar then forms
    out = c * r. Order scan_A, TS_A, scan_B, TS_B overlaps TS_A and the
    A-rows' output DMA with phase B's arrival and scan.
  - DMA back to HBM as (256, 64): SP ships the A rows early, ACT ships
    the B rows after TS_B.

Numerics note: the reference's f32 forward pass underflows to exactly
0.0 everywhere (the activation scale squares at every level:
1e-1 -> 1e-2 -> 1e-4 -> ... -> ~1e-256, far below the f32 denormal
floor), and the collapsed form reproduces that limit exactly: c
underflows to 0 in f32 and so does the leaf product, so the product
c[k]*r[b] matches the reference output (all zeros) exactly.
"""

import sys
import types

import numpy as np

import concourse.bass as bass
from concourse import mybir
from concourse.bass_utils import run_bass_kernel_spmd


def _ensure_ntff_hook() -> None:
    """Best-effort: provide ``antenv.axon_hooks`` when the image lacks it.

    ``run_bass_kernel_spmd(trace=True)`` (or BASS_TRACE=1 in the env)
    imports ``antenv.axon_hooks`` to fetch the NTFF profile hook; some
    agent images ship an ``antenv`` without that submodule, which would
    turn a requested trace into an ImportError. Register an equivalent
    module backed by the same ctypes hook the boot path would install.
    No-op if the real module exists or anything is missing.
    """
    try:
        import antenv.axon_hooks  # noqa: F401

        return
    except ImportError:
        pass
    try:
        import antenv
        from trn_agent_boot.trn_boot import _ntff_profile_via_ctypes

        hook = _ntff_profile_via_ctypes("/opt/axon/libaxon_pjrt.so")
        mod = types.ModuleType("antenv.axon_hooks")
        _state = {"hook": hook}
        mod.set_axon_ntff_profile_hook = lambda h: _state.__setitem__("hook", h)
        mod.get_axon_ntff_profile_hook = lambda: _state["hook"]
        sys.modules["antenv.axon_hooks"] = mod
        antenv.axon_hooks = mod
    except Exception:
        pass

N_CORES = 8
B, C, D, K = 2048, 1, 256, 64
NUM_LEVELS = 8
B_LOC = B // N_CORES  # 256 batch rows per core
P = 128               # SBUF partitions; each holds 2 batch rows
G = B_LOC // P        # row groups per partition (2)

# Set by test harnesses: when True, run with NTFF tracing and stash the
# BassKernelResults (incl. exec_time_ns) in LAST_RESULT.
TRACE = False
LAST_RESULT = None

_NC_CACHE = None


def _build_bass() -> bass.Bass:
    """(128, 2x256) x slab -> row-product scans -> scale by c -> (256, 64).

    Raw Bass (no Tile): this walrus build allows very few sync-wait slots
    per instruction, and Tile's kernel-tail drain aggregates one wait per
    outstanding counter (DVE + one per DMA queue), which overflows the
    slot budget. With explicit semaphores every instruction carries at
    most one wait.

    Layout: partition p holds batch rows p (phase/group A) and 128+p
    (phase B) as contiguous HBM half-slabs; both phases are striped
    across the two HWDGE engines (SP, ACT). Phase A rows additionally
    carry the 64-wide c vector (1280 B lines), so c lands with asem and
    needs no DMA of its own. Phase B (1024 B lines) follows on the same
    queues.
    DVE order is scan_A, TS_A, scan_B, TS_B: TS_A executes in the gap
    where the DVE would otherwise idle waiting for phase B, letting SP
    ship the A-rows' output while scan_B still runs; ACT ships the
    B-rows' output last. vsem counts DVE ops in program order, so each
    single wait slot encodes its full dependency set transitively.
    """
    nc = bass.Bass(use_seq_codegen=True)
    xga = nc.declare_dram_parameter("xga", [P, D + K], mybir.dt.float32, isOutput=False)
    xgb = nc.declare_dram_parameter("xgb", [P, D], mybir.dt.float32, isOutput=False)
    out = nc.declare_dram_parameter("out", [B_LOC, K], mybir.dt.float32, isOutput=True)

    with (
        nc.sbuf_tensor([P, D + K], mybir.dt.float32) as xta,
        nc.sbuf_tensor([P, D], mybir.dt.float32) as xtb,
        nc.sbuf_tensor([P, D // 2], mybir.dt.float32) as ra,
        nc.sbuf_tensor([P, D // 2], mybir.dt.float32) as rb,
        nc.sbuf_tensor([P, G * K], mybir.dt.float32) as ot,
        nc.semaphore("dsem") as dsem,
        nc.semaphore("asem") as asem,
        nc.semaphore("bsem") as bsem,
        nc.semaphore("vsem") as vsem,
        nc.Block() as block,
    ):
        H = P // 2     # partition stripe per HWDGE engine (keep stripes
        #                64-row aligned: uneven splits inflate the DMA
        #                issue time ~2x, measured)
        DTOT = 16 * 2  # out-A + out-B on dsem
        # vsem counts completed DVE ops (program order scan_A, TS_A,
        # scan_B, TS_B). scan_B carries no vsem wait (disjoint output),
        # but cannot complete before TS_A (in-order engine), so >= 2
        # implies TS_A done and >= 4 implies everything done.
        V_OUTA, NV_END = 2, 4

        def io_stream(eng, sl, g, vwait):
            # One HWDGE engine moves its partition stripe of phase A
            # (batch rows 0:128 + the c tail) then phase B (rows
            # 128:256), and on the way out ships one row group for all
            # partitions: SP ships the A rows as soon as TS_A is done
            # (while scan_B/TS_B still run), ACT ships the B rows after
            # TS_B. Row groups are contiguous half-slabs, so every DMA's
            # HBM side is linear (aggregation-friendly bursts).
            eng.dma_start(out=xta[sl, :], in_=xga[sl, :]).then_inc(asem, 16)
            eng.dma_start(out=xtb[sl, :], in_=xgb[sl, :]).then_inc(bsem, 16)
            eng.wait_ge(vsem, vwait)
            eng.dma_start(
                out=out[g * P : (g + 1) * P, :], in_=ot[:, g * K : (g + 1) * K]
            ).then_inc(dsem, 16)
            eng.wait_ge(dsem, DTOT)

        @block.sync
        def _(sync):
            io_stream(sync, slice(0, H), g=0, vwait=V_OUTA)

        @block.scalar
        def _(scalar):
            io_stream(scalar, slice(H, P), g=1, vwait=NV_END)

        @block.vector
        def _(vector):
            # Per-row product via one cumulative-product scan per row
            # group:  state = (a[t] * state) * b[t]  with a = the row's
            # first half, b = its second half, so a 128-step scan yields
            # the full 256-leaf product in its last column. DVE op N+1
            # reading op N's output needs a semaphore (measured on HW:
            # without one the read races the writeback), so each
            # dependent op waits on vsem riding the op instruction.
            h = D // 2

            def scan(xt, r, dma_sem):
                ins = nc.vector.tensor_tensor_scan(
                    out=r[:, :],
                    data0=xt[:, 0:h],
                    data1=xt[:, h:D],
                    initial=1.0,
                    op0=mybir.AluOpType.mult,
                    op1=mybir.AluOpType.mult,
                )
                ins._wait_ge(dma_sem, 32)
                ins.then_inc(vsem, 1)

            def scale(g, r, vwait):
                # out[p, g, kk] = c[kk] * r[p]; r = the scan's last column
                ins = nc.vector.tensor_scalar(
                    out=ot[:, g * K : (g + 1) * K],
                    in0=xta[:, D : D + K],
                    scalar1=r[:, h - 1 : h],
                    scalar2=None,
                    op0=mybir.AluOpType.mult,
                )
                ins._wait_ge(vsem, vwait)
                ins.then_inc(vsem, 1)

            scan(xta, ra, asem)
            scale(0, ra, 1)   # after scan_A (c arrived with asem)
            scan(xtb, rb, bsem)
            scale(1, rb, 3)   # after scan_B (and, in order, TS_A)

    return nc


def _get_bass() -> bass.Bass:
    global _NC_CACHE
    if _NC_CACHE is None:
        _NC_CACHE = _build_bass()
    return _NC_CACHE


def _fold_weights(inputs: dict) -> np.ndarray:
    """Run the weight-only u-recursion (f64) down to the root: c = u_8[0]."""
    u = np.asarray(inputs["w_in"], dtype=np.float64)[:, :, 0]  # (D, K), C == 1
    for l in range(NUM_LEVELS):
        idx = np.asarray(inputs[f"idx{l}"], dtype=np.int64)
        w = np.asarray(inputs[f"w{l}"], dtype=np.float64)
        u = np.einsum("foi,fi->fo", w, u[idx[:, 0]] * u[idx[:, 1]])
    return u[0].astype(np.float32)  # (K,)


def kernel(**inputs: np.ndarray) -> np.ndarray:
    x = np.asarray(inputs["x"], dtype=np.float32)          # (B, 1, D)
    scope = np.asarray(inputs["scope_idx"], dtype=np.int64)[:, 0]

    c = _fold_weights(inputs)                               # (K,) f32

    # Input-layer bookkeeping gather (leaf scope of the root's product).
    xg = x[:, 0, :][:, scope]                               # (B, D)

    # Per core: phase A = batch rows 0:128 with c appended (so c rides
    # the same DMA), phase B = rows 128:256.
    _ensure_ntff_hook()
    nc = _get_bass()
    in_maps = []
    for i in range(N_CORES):
        sl = xg[i * B_LOC : (i + 1) * B_LOC]
        xga = np.empty((P, D + K), dtype=np.float32)
        xga[:, :D] = sl[:P]
        xga[:, D:] = c[None, :]
        in_maps.append({"xga": xga, "xgb": np.ascontiguousarray(sl[P:])})
    res = run_bass_kernel_spmd(
        nc, in_maps, list(range(N_CORES)), trace=TRACE, trace_cores=[0] if TRACE else None
    )
    global LAST_RESULT
    LAST_RESULT = res

    out = np.concatenate([res.results[i]["out"] for i in range(N_CORES)], axis=0)
    return np.ascontiguousarray(out.reshape(B, C, K))
